# revision 24
# baseline (speedup 1.0000x reference)
"""Multi-head causal attention with RoPE on 8 TRN2 NeuronCores.

Problem: B=2, T=2048, D=1024, H=16 heads (dh=64), fp32 I/O.
  q/k/v = x @ w{q,k,v}.T ; RoPE(q,k) ; causal softmax((q k^T)/sqrt(dh)) @ v ;
  out = concat_heads @ wo.T

Sharding (8 cores): head-parallel compute, token-striped output. Core c owns
heads {2c, 2c+1} for both batches; four AllToAll collectives redistribute
normalized attention outputs so core c applies the full output projection for
token chunks {c, c+8, c+16, c+24}. The host interleaves chunks back.

v2 scheduling notes (the perf-critical part):
 - The PE HAM clock gate re-throttles to 1.2 GHz whenever the PE micro-idles;
   the attention QK->exp->PV chain alone leaves ~400ns holes per chunk. All
   projection / output-projection matmuls are therefore fed through a FIFO
   "filler" queue and interleaved at chunk granularity, paced by an emission
   -time estimator that keeps the in-order PE queue ahead of the exp stream.
 - The softmax epilogue is kept off the PE critical path: denominator rows are
   gathered by tiny DMAs on the (otherwise idle) gpsimd queue, inverted with
   reciprocal_approx_fast (5x cheaper than InstReciprocal), and only then do
   the small PE broadcast matmuls run, after a pump of independent PE work.
 - DMA triggers live only on the sync + gpsimd queues (a DMA occupies its
   issuing engine queue for the whole transfer, so scalar/vector-queue DMAs
   would stall the activation/DVE engines).
 - V is projected with x as the stationary operand so it lands token-major
   ([tok, feat]) straight into PSUM: no PE transposes, no psum->sbuf bounce.
 - Startup: first xT block arrives in per-2-chunk pieces so the first
   projection matmul issues ~2us in; b=1 runs (1,1),(1,2),(1,3),(1,0) so the
   final, collective-exposed attention unit is the 4-chunk one.
 - Softmax skips the running-max: |scores|/8 < ~6 for unit-variance inputs,
   exp is computed in fp32 and cannot overflow.
"""

from collections import deque

import numpy as np
import ml_dtypes

import concourse.bacc as bacc
import concourse.tile as tile
import concourse.mybir as mybir
from concourse import bass_utils

BF16 = mybir.dt.bfloat16
F32 = mybir.dt.float32
AF = mybir.ActivationFunctionType

NCORES = 8
B, T, D, H = 2, 2048, 1024, 16
DH = D // H          # 64
HPC = H // NCORES    # 2 heads per core
FPC = DH * HPC       # 128 features per core
TOK = B * T          # 4096
TPC = TOK // NCORES  # 512 tokens per core (output shard)
KC = D // 128        # 8 contraction chunks
NT = T // 512        # 4 query tiles of 512 per batch
VG = 132             # cols per v-slot: [v_h0(64) | 1 | v_h1(64) | 1 | pad2]

_COMPILED = None

# emission-time pacing model (ns); deliberately pessimistic on PE speed so the
# pump over-feeds rather than starves
MM_NS = 0.45   # per moving column
MM_FIX = 70.0
EXP_NS = 0.84  # per column on ACT
EXP_FIX = 350.0
SEM_NS = 200.0


def _build(debug_taps=False):
    nc = bacc.Bacc("TRN2", target_bir_lowering=False, debug=False, num_devices=NCORES)

    xT_d = nc.dram_tensor("xT", [D, TOK], BF16, kind="ExternalInput")
    wq_d = nc.dram_tensor("wqT", [D, FPC], BF16, kind="ExternalInput")
    wk_d = nc.dram_tensor("wkT", [D, FPC], BF16, kind="ExternalInput")
    wv_d = nc.dram_tensor("wvT", [D, FPC], BF16, kind="ExternalInput")
    wo_d = nc.dram_tensor("woT", [D, D], BF16, kind="ExternalInput")
    C_d = nc.dram_tensor("cosC", [128, T], BF16, kind="ExternalInput")
    S_d = nc.dram_tensor("sinS", [128, T], BF16, kind="ExternalInput")
    mask_d = nc.dram_tensor("mask", [128, 128], BF16, kind="ExternalInput")
    sel_d = nc.dram_tensor("sel", [4, 4 * DH], BF16, kind="ExternalInput")
    out_d = nc.dram_tensor("out", [TPC, D], F32, kind="ExternalOutput")

    swap16 = list(range(16, 32)) + list(range(16))

    with tile.TileContext(nc) as tc:
        with (
            tc.tile_pool(name="sb", bufs=1) as sb,
            tc.tile_pool(name="ps", bufs=1, space="PSUM") as ps,
            tc.tile_pool(name="dram", bufs=1, space="DRAM") as dram,
        ):
            # ---- persistent SBUF tiles ----
            wq_sb = sb.tile([128, KC * FPC], BF16)
            wk_sb = sb.tile([128, KC * FPC], BF16)
            wv_sb = sb.tile([128, KC * FPC], BF16)
            C_sb = sb.tile([128, T], BF16)
            S_sb = sb.tile([128, T], BF16)
            mask2_sb = sb.tile([128, 256], BF16)
            sel_sb = sb.tile([4, 4 * DH], BF16)
            xT_sb = sb.tile([128, KC * TOK], BF16)
            wo_sb = sb.tile([128, KC * D], BF16)
            qrot_sb = sb.tile([128, TOK], BF16)
            krot_sb = sb.tile([128, TOK], BF16)
            v1_sb = sb.tile([128, B * (T // 128) * VG], BF16)

            # ones columns of the v-slots (softmax denominator trick) --
            # emitted first so the gpsimd engine isn't busy with transfers
            # when the first v copies need it
            v1v = v1_sb[:].rearrange("p (g c) -> p g c", c=VG)
            nc.gpsimd.memset(v1v[:, :, 64:65], 1.0)
            nc.gpsimd.memset(v1v[:, :, 129:130], 1.0)

            # ---- input loads ----
            # gpsimd queue: small latency-critical tensors, in need-order
            def load_w_half(w_sb, w_d, h, quarters=1):
                step = 4 // quarters
                for s in range(quarters):
                    k0 = 4 * h + step * s
                    nc.gpsimd.dma_start(
                        w_sb[:, k0 * FPC : (k0 + step) * FPC].rearrange(
                            "p (k c) -> p k c", k=step
                        ),
                        w_d[k0 * 128 : (k0 + step) * 128, :].rearrange(
                            "(k p) c -> p k c", p=128
                        ),
                    )

            load_w_half(wq_sb, wq_d, 0, quarters=4)
            load_w_half(wq_sb, wq_d, 1)
            load_w_half(wk_sb, wk_d, 0)
            load_w_half(wk_sb, wk_d, 1)
            nc.gpsimd.dma_start(C_sb[:, 0:512], C_d[:, 0:512])
            nc.gpsimd.dma_start(S_sb[:, 0:512], S_d[:, 0:512])
            load_w_half(wv_sb, wv_d, 0)
            load_w_half(wv_sb, wv_d, 1)
            nc.gpsimd.dma_start(mask2_sb[:, 0:128], mask_d[:])
            nc.gpsimd.dma_start(mask2_sb[:, 128:256], mask_d[:])
            nc.gpsimd.dma_start(sel_sb[:], sel_d[:])
            nc.gpsimd.dma_start(C_sb[:, 512:T], C_d[:, 512:T])
            nc.gpsimd.dma_start(S_sb[:, 512:T], S_d[:, 512:T])

            # sync queue: xT token-blocks ((b,n) spans KC*512 cols, k-chunk
            # major inside). First block arrives in 4 kc-pair pieces so the
            # first projection matmul can issue after ~128KB.
            def load_x_piece(b, n, k0, k1):
                col = b * T + 512 * n
                blk = (NT * b + n) * (KC * 512)
                nc.sync.dma_start(
                    xT_sb[:, blk + 512 * k0 : blk + 512 * k1].rearrange(
                        "p (k t) -> p k t", k=k1 - k0
                    ),
                    xT_d[128 * k0 : 128 * k1, col : col + 512].rearrange(
                        "(k p) t -> p k t", p=128
                    ),
                )

            load_x_piece(0, 0, 0, 1)
            load_x_piece(0, 0, 1, 2)
            for k0 in range(2, 8, 2):
                load_x_piece(0, 0, k0, k0 + 2)
            for k0 in range(0, 8, 4):
                load_x_piece(0, 1, k0, k0 + 4)
            for b, n in ((1, 0), (1, 1), (0, 2), (0, 3), (1, 2), (1, 3)):
                load_x_piece(b, n, 0, 8)
            nc.sync.dma_start(
                wo_sb[:].rearrange("p (k c) -> p k c", k=KC),
                wo_d[:].rearrange("(k p) c -> p k c", p=128),
            )

            # 4 AllToAll groups: group g carries global token chunk 8g+o to rank o
            a2a_in = [dram.tile([D, 128], BF16, name=f"a2ain{g}") for g in range(4)]
            a2a_out = [dram.tile([D, 128], BF16, name=f"a2aout{g}") for g in range(4)]

            # ---- filler queue + pacing ----
            # entries: [block_key, cost_ns, emit_fn, earliest_pe_clk]
            # `earliest` gates collective-dependent fillers (output-projection
            # matmuls) so the pump can never park an a2a-blocked matmul in
            # front of latency-critical PE work.
            fillers = []
            clk = {"pe": 0.0, "act": 0.0}
            done_blocks = set()

            def emit_at(idx):
                key, cost, fn, _ = fillers.pop(idx)
                fn()
                clk["pe"] += cost
                if key is not None and not any(e[0] == key for e in fillers):
                    done_blocks.add(key)

            def first_eligible():
                for i, e in enumerate(fillers):
                    if e[3] <= clk["pe"]:
                        return i
                return None

            # keep ~8us of eligible filler work in reserve for the final
            # collective's wait window (PE stays warm into the last output
            # projection); drain_all / drain_block ignore the reserve
            RESERVE = {"ns": 0.0}

            def pump_until(t_ns):
                while clk["pe"] < t_ns:
                    elig = [e for e in fillers if e[3] <= clk["pe"]]
                    if not elig or sum(e[1] for e in elig) <= RESERVE["ns"]:
                        return
                    i = first_eligible()
                    emit_at(i)

            def drain_block(key):
                while key not in done_blocks:
                    i = next(
                        (i for i, e in enumerate(fillers) if e[0] == key), None
                    )
                    if i is None:
                        done_blocks.add(key)
                        return
                    emit_at(i)

            def drain_all():
                while fillers:
                    emit_at(0)

            # ---- projection emitters (as fillers) ----
            def qk_proj_block(w_sb, dst_sb, b, n):
                """q or k projection for token block (b,n): 8 MMs into one
                psum tile, then RoPE into dst_sb. Emitted as 4 fillers. The
                psum tile is allocated inside the first filler so pool-buffer
                cycling follows emission order, not enqueue order."""
                blk = (NT * b + n) * (KC * 512)
                hold = {}

                def mms(k0):
                    def fn():
                        if k0 == 0:
                            hold["pp"] = ps.tile(
                                [128, 512], F32, tag="proj", bufs=2,
                                name=f"pp{b}{n}{'q' if dst_sb is qrot_sb else 'k'}",
                            )
                        pp = hold["pp"]
                        for kc in range(k0, k0 + 2):
                            nc.tensor.matmul(
                                pp[:],
                                w_sb[:, kc * FPC : (kc + 1) * FPC],
                                xT_sb[:, blk + 512 * kc : blk + 512 * kc + 512],
                                start=(kc == 0),
                                stop=(kc == KC - 1),
                            )
                        if k0 == 6:
                            rope(pp, dst_sb, b, n)
                            del hold["pp"]
                    return fn

                for k0 in range(0, 8, 2):
                    yield 2 * (512 * MM_NS + MM_FIX), mms(k0)

            def rope(pp, dst_sb, b, n):
                # dst = pp*C + swap16(pp)*S, reading the projection psum
                swp = sb.tile([128, 512], F32, tag="swp", bufs=2, name=f"swp{b}{n}")
                nc.vector.stream_shuffle(swp[:], pp[:], swap16)
                t1 = sb.tile([128, 512], BF16, tag="t1", bufs=2, name=f"t1{b}{n}")
                nc.vector.tensor_mul(t1[:], pp[:], C_sb[:, 512 * n : 512 * n + 512])
                t2 = sb.tile([128, 512], BF16, tag="t2", bufs=2, name=f"t2{b}{n}")
                nc.vector.tensor_mul(t2[:], swp[:], S_sb[:, 512 * n : 512 * n + 512])
                nc.vector.tensor_add(
                    dst_sb[:, b * T + 512 * n : b * T + 512 * n + 512], t1[:], t2[:]
                )

            def v_proj_block(b, n):
                """V projection token-major: stationary = xT chunk, moving =
                wv chunk, psum [128 tok, 128 feat]; DVE copies the two head
                halves into the v-slot (with its ones columns). 4 fillers."""
                blk = (NT * b + n) * (KC * 512)

                def one_tc(tc_i):
                    def fn():
                        vps = ps.tile([128, 128], F32, tag="proj", bufs=2,
                                      name=f"vps{b}{n}{tc_i}")
                        for kc in range(KC):
                            nc.tensor.matmul(
                                vps[:],
                                xT_sb[:, blk + 512 * kc + 128 * tc_i :
                                      blk + 512 * kc + 128 * tc_i + 128],
                                wv_sb[:, kc * FPC : (kc + 1) * FPC],
                                start=(kc == 0),
                                stop=(kc == KC - 1),
                            )
                        g = VG * ((T // 128) * b + 4 * n + tc_i)
                        nc.vector.tensor_copy(
                            v1_sb[:, g : g + 130].rearrange(
                                "p (h c) -> p h c", h=2
                            )[:, :, 0:64],
                            vps[:].rearrange("p (h c) -> p h c", h=2),
                        )
                    return fn

                for tc_i in range(4):
                    yield 8 * (128 * MM_NS + MM_FIX), one_tc(tc_i)

            def enqueue_block(b, n):
                # separate keys: unit (b,j) needs rope-q at its first chunk
                # but rope-k / v1 chunks only at its last four (diagonal)
                # chunks, so those parts can keep serving as pump fillers
                for cost, fn in qk_proj_block(wq_sb, qrot_sb, b, n):
                    fillers.append([("P", b, n, "q"), cost, fn, 0.0])
                for cost, fn in qk_proj_block(wk_sb, krot_sb, b, n):
                    fillers.append([("P", b, n, "k"), cost, fn, 0.0])
                for cost, fn in v_proj_block(b, n):
                    fillers.append([("P", b, n, "v"), cost, fn, 0.0])

            # ---- attention ----
            def attn_unit(b, j, nxt=None):
                """Both heads for (batch b, q-tile j). The two heads' S tiles
                share one 2-bank psum pair; QK pairs run in disjoint PE row
                groups; PV trails QK by one chunk; fillers are pumped between
                chunks so the in-order PE queue never drains while ACT runs
                exp. `nxt` names the following unit so its rope-q gets forced
                out early (DVE queue slack). Returns the two [65,512] f32
                SBUF copies of the O accumulators (row 64 = softmax sums)."""
                drain_block(("P", b, j, "q"))
                if j == 0:
                    drain_block(("P", b, j, "k"))
                    drain_block(("P", b, j, "v"))
                ops = [
                    ps.tile([65, 512], F32, tag=f"oph{h}", bufs=1, name=f"op{b}{h}{j}")
                    for h in range(2)
                ]
                nch = 4 * j + 4

                def qk_exp(c):
                    diag = c - 4 * j
                    lo = 128 * diag if diag >= 0 else 0
                    sp = ps.tile([128, 1024], F32, tag="spsum", bufs=2,
                                 name=f"sp{b}{j}{c}")
                    spv = sp[:].rearrange("p (h t) -> p h t", h=2)
                    for h in range(2):
                        nc.tensor.matmul(
                            sp[:, 512 * h + lo : 512 * h + 512],
                            krot_sb[64 * h : 64 * h + 64,
                                    b * T + 128 * c : b * T + 128 * c + 128],
                            qrot_sb[64 * h : 64 * h + 64,
                                    b * T + 512 * j + lo : b * T + 512 * j + 512],
                            start=True,
                            stop=True,
                        )
                    cols = 512 - lo
                    clk["pe"] += cols * MM_NS + MM_FIX + 50
                    pt = sb.tile([128, 1024], BF16, tag="pt", bufs=4,
                                 name=f"pt{b}{j}{c}")
                    ptv = pt[:].rearrange("p (h t) -> p h t", h=2)
                    nc.scalar.activation(
                        ptv[:, :, lo:512], spv[:, :, lo:512], AF.Exp, scale=0.125
                    )
                    clk["act"] = max(clk["act"], clk["pe"] + SEM_NS) + (
                        2 * cols * EXP_NS + EXP_FIX
                    )
                    t_ready = clk["act"]
                    if diag >= 0:
                        nc.vector.tensor_mul(
                            ptv[:, :, lo : lo + 128], ptv[:, :, lo : lo + 128],
                            mask2_sb[:].rearrange("p (h t) -> p h t", h=2),
                        )
                        t_ready += 350
                    return pt, t_ready

                def pv(c, pt):
                    diag = c - 4 * j
                    lo = 128 * diag if diag >= 0 else 0
                    g = VG * ((T // 128) * b + c)
                    for h in range(2):
                        nc.tensor.matmul(
                            ops[h][:, lo:512],
                            v1_sb[:, g + 65 * h : g + 65 * h + 65],
                            pt[:, 512 * h + lo : 512 * h + 512],
                            start=(c == 0),
                            stop=(c == nch - 1),
                        )
                    clk["pe"] += 2 * ((512 - lo) * MM_NS + MM_FIX)

                prev = None
                for c in range(nch):
                    if c == max(1, 4 * j - 4):
                        drain_block(("P", b, j, "k"))
                        drain_block(("P", b, j, "v"))
                    if c == min(2, nch - 1) and nxt is not None:
                        drain_block(("P", nxt[0], nxt[1], "q"))
                    cur, t_ready = qk_exp(c)
                    if prev is not None:
                        pump_until(prev[1])
                        clk["pe"] = max(clk["pe"], prev[1])
                        pv(c - 1, prev[0])
                    prev = (cur, t_ready)
                pump_until(prev[1])
                clk["pe"] = max(clk["pe"], prev[1])
                pv(nch - 1, prev[0])
                # copy accumulators to SBUF promptly (opsum bufs=1): h0 on
                # ACT, h1 on DVE so the boundary costs ~one copy not two
                o65s = []
                for h in range(2):
                    o65 = sb.tile([65, 512], F32, tag="o65", bufs=8,
                                  name=f"o65{b}{h}{j}")
                    if h == 0:
                        nc.scalar.activation(o65[:], ops[h][:], AF.Copy)
                        clk["act"] += 512 * EXP_NS + EXP_FIX
                    else:
                        nc.vector.tensor_copy(o65[:], ops[h][:])
                    o65s.append(o65)
                return o65s

            def attn_epilogue(b, js, o65_by_j, pump=1200):
                """Normalize one or two q-tiles' outputs with one batched
                fast-reciprocal, broadcast via small PE matmuls (emitted after
                a pump of independent PE work), then stage into the a2a
                buffer with one DMA per q-tile."""
                tag = f"{b}{js[0]}"
                nu = 2 * len(js)
                sums = sb.tile([4, 512], F32, tag="sums", bufs=2, name=f"sums{tag}")
                units = [(j, h, o65_by_j[j][h]) for j in js for h in range(2)]
                for r, (j, h, o65) in enumerate(units):
                    nc.gpsimd.dma_start(sums[r : r + 1, :], o65[64:65, :])
                rec4 = sb.tile([4, 512], F32, tag="rec4", bufs=2, name=f"rec4{tag}")
                nc.vector.reciprocal_approx_fast(rec4[0:nu, :], sums[0:nu, :])
                recb4 = sb.tile([4, 512], BF16, tag="recb4", bufs=2, name=f"recb4{tag}")
                nc.vector.tensor_copy(recb4[0:nu, :], rec4[0:nu, :])
                if pump:
                    pump_until(clk["pe"] + pump)
                onrs = {}
                for j in js:
                    onrs[j] = sb.tile([128, 512], BF16, tag="onr", bufs=4,
                                      name=f"onr{b}{j}")
                for r, (j, h, o65) in enumerate(units):
                    bps = ps.tile([64, 512], F32, tag="proj", bufs=2,
                                  name=f"bps{b}{j}{h}")
                    nc.tensor.matmul(
                        bps[:], sel_sb[0:nu, DH * r : DH * r + DH], recb4[0:nu, :],
                        start=True, stop=True,
                    )
                    clk["pe"] += 512 * MM_NS + MM_FIX
                    nc.vector.tensor_mul(
                        onrs[j][64 * h : 64 * h + 64, :], o65[0:64, :], bps[:]
                    )
                for j in js:
                    m0 = 16 * b + 4 * j
                    o0, g = m0 % 8, m0 // 8
                    nc.gpsimd.dma_start(
                        a2a_in[g][:].rearrange("(o p) c -> p o c", p=128)[
                            :, o0 : o0 + 4, :
                        ],
                        onrs[j][:].rearrange("p (i c) -> p i c", c=128),
                    )

            trigger_clk = {}

            def a2a_call(g):
                nc.gpsimd.collective_compute(
                    "AllToAll",
                    mybir.AluOpType.bypass,
                    replica_groups=[list(range(NCORES))],
                    ins=[a2a_in[g].opt()],
                    outs=[a2a_out[g].opt()],
                )
                trigger_clk[g] = clk["pe"]

            # ---- output projection (as fillers) ----
            def enqueue_final(g):
                key = ("F", g)
                fps = {}

                def at_load(half):
                    def fn():
                        if half == 0:
                            fps["at"] = sb.tile(
                                [128, KC * 128], BF16, tag="at", bufs=2, name=f"at{g}"
                            )
                        # sync queue: an at-load can stall on the collective;
                        # on the gpsimd queue that would block later sums /
                        # staging DMAs and the next collective trigger
                        nc.sync.dma_start(
                            fps["at"][:, 512 * half : 512 * half + 512].rearrange(
                                "p (k c) -> p k c", k=4
                            ),
                            a2a_out[g][512 * half : 512 * half + 512, :].rearrange(
                                "(k p) c -> p k c", p=128
                            ),
                        )
                    return fn

                def mms(nh, k0):
                    def fn():
                        if k0 == 0:
                            fps[nh] = ps.tile([128, 512], F32, tag="proj", bufs=2,
                                              name=f"fp{g}{nh}")
                        fp = fps[nh]
                        at = fps["at"]
                        for kc in range(k0, k0 + 2):
                            nc.tensor.matmul(
                                fp[:],
                                at[:, 128 * kc : 128 * kc + 128],
                                wo_sb[:, kc * D + 512 * nh : kc * D + 512 * nh + 512],
                                start=(kc == 0),
                                stop=(kc == KC - 1),
                            )
                        if k0 == 6:
                            fo = sb.tile([128, 512], F32, tag="fo", bufs=2,
                                         name=f"fo{g}{nh}")
                            nc.vector.tensor_copy(fo[:], fp[:])
                            nc.sync.dma_start(
                                out_d[128 * g : 128 * g + 128,
                                      512 * nh : 512 * nh + 512],
                                fo[:],
                            )
                    return fn

                earliest = trigger_clk[g] + 22000.0
                fillers.append([key, 0, at_load(0), earliest])
                fillers.append([key, 0, at_load(1), earliest])
                # nh-interleaved kc halves so the second at-load overlaps
                for k0 in (0, 2, 4, 6):
                    for nh in range(2):
                        fillers.append(
                            [key, 2 * (512 * MM_NS + MM_FIX), mms(nh, k0), earliest]
                        )

            # ---- schedule ----
            # Batches interleaved small-units-first so exp-bound attention
            # chunks are spread across the whole filler supply, and the last
            # collective (g3) fires as early as the data flow allows.
            # Collective order: g0, g2, g1, g3.
            for bn in ((0, 0), (0, 1)):
                enqueue_block(*bn)
                for part in ("q", "k", "v"):
                    drain_block(("P", bn[0], bn[1], part))
            for bn in ((1, 0), (1, 1), (0, 2), (0, 3), (1, 2), (1, 3)):
                enqueue_block(*bn)

            o0, o1 = {}, {}
            o0[0] = attn_unit(0, 0, nxt=(0, 1))
            o0[1] = attn_unit(0, 1, nxt=(1, 0))
            attn_epilogue(0, (0, 1), o0)
            a2a_call(0)
            o1[0] = attn_unit(1, 0, nxt=(1, 1))
            o1[1] = attn_unit(1, 1, nxt=(0, 2))
            attn_epilogue(1, (0, 1), o1)
            a2a_call(2)
            enqueue_final(0)
            o0[2] = attn_unit(0, 2, nxt=(0, 3))
            enqueue_final(2)
            o0[3] = attn_unit(0, 3, nxt=(1, 2))
            attn_epilogue(0, (2, 3), o0)
            a2a_call(1)
            RESERVE["ns"] = 8000.0
            o1[2] = attn_unit(1, 2, nxt=(1, 3))
            attn_epilogue(1, (2,), o1)
            enqueue_final(1)
            o1[3] = attn_unit(1, 3)
            attn_epilogue(1, (3,), o1, pump=800)
            a2a_call(3)
            enqueue_final(3)
            drain_all()

            if debug_taps:
                dbg_q = nc.dram_tensor("dbg_q", [128, TOK], BF16, kind="ExternalOutput")
                dbg_k = nc.dram_tensor("dbg_k", [128, TOK], BF16, kind="ExternalOutput")
                dbg_v1 = nc.dram_tensor(
                    "dbg_v1", [128, B * (T // 128) * VG], BF16, kind="ExternalOutput"
                )
                dbg_a2a = nc.dram_tensor("dbg_a2a", [D, TPC], BF16, kind="ExternalOutput")
                nc.sync.dma_start(dbg_q[:], qrot_sb[:])
                nc.sync.dma_start(dbg_k[:], krot_sb[:])
                nc.sync.dma_start(dbg_v1[:], v1_sb[:])
                for g in range(4):
                    nc.sync.dma_start(dbg_a2a[:, 128 * g : 128 * g + 128], a2a_in[g][:])

    nc.compile()
    return nc


def _get_compiled():
    global _COMPILED
    if _COMPILED is None:
        _COMPILED = _build()
    return _COMPILED


def _prep_in_maps(embedding_word, wq, wk, wv, wo):
    bf = ml_dtypes.bfloat16
    x = np.asarray(embedding_word, np.float32).reshape(TOK, D)
    xT = np.ascontiguousarray(x.T).astype(bf)
    woT = np.ascontiguousarray(np.asarray(wo, np.float32).T).astype(bf)

    # within-head row permutation: 16 re rows then 16 im rows per 32-row quadrant
    perm64 = [
        (2 * (16 * q + r) if r < 16 else 2 * (16 * q + (r - 16)) + 1)
        for q in range(2)
        for r in range(32)
    ]
    perm64 = np.asarray(perm64)

    freqs = 1.0 / (10000.0 ** (np.arange(0, DH, 2, dtype=np.float64) / DH))  # [32]
    ang = np.arange(T, dtype=np.float64)[:, None] * freqs[None, :]  # [T, 32]
    cos_t, sin_t = np.cos(ang), np.sin(ang)
    rows = np.arange(128)
    wh = rows % 64
    qd = wh // 32
    r32 = wh % 32
    dmap = 16 * qd + (r32 % 16)
    sign = np.where(r32 < 16, -1.0, 1.0)
    C = np.ascontiguousarray(cos_t[:, dmap].T).astype(bf)  # [128, T]
    S = np.ascontiguousarray((sin_t[:, dmap] * sign[None, :]).T).astype(bf)

    rr = np.arange(128)[:, None]
    cc = np.arange(128)[None, :]
    mask = np.where(cc >= rr, 1.0, 0.0).astype(ml_dtypes.bfloat16)
    sel = np.zeros((4, 4 * DH), np.float32)
    for r in range(4):
        sel[r, DH * r : DH * r + DH] = 1.0
    sel = sel.astype(bf)

    wqf = np.asarray(wq, np.float32)
    wkf = np.asarray(wk, np.float32)
    wvf = np.asarray(wv, np.float32)

    in_maps = []
    for c in range(NCORES):
        rows_c = slice(FPC * c, FPC * c + FPC)
        wq_c = wqf[rows_c].reshape(HPC, DH, D)[:, perm64, :].reshape(FPC, D)
        wk_c = wkf[rows_c].reshape(HPC, DH, D)[:, perm64, :].reshape(FPC, D)
        wv_c = wvf[rows_c]
        in_maps.append(
            {
                "xT": xT,
                "wqT": np.ascontiguousarray(wq_c.T).astype(bf),
                "wkT": np.ascontiguousarray(wk_c.T).astype(bf),
                "wvT": np.ascontiguousarray(wv_c.T).astype(bf),
                "woT": woT,
                "cosC": C,
                "sinS": S,
                "mask": mask,
                "sel": sel,
            }
        )
    return in_maps


def _unshard(core_outs):
    """core_outs[c] is [TPC, D] covering token chunks {c, 8+c, 16+c, 24+c}
    (row-blocks g=0..3). Interleave back to [B, T, D]."""
    a = np.stack(core_outs, axis=0)  # [8, TPC, D]
    a = a.reshape(NCORES, 4, 128, D).transpose(1, 0, 2, 3).reshape(TOK, D)
    return np.ascontiguousarray(a.reshape(B, T, D).astype(np.float32))


def kernel(embedding_word, wq, wk, wv, wo):
    nc = _get_compiled()
    in_maps = _prep_in_maps(embedding_word, wq, wk, wv, wo)
    res = bass_utils.run_bass_kernel_spmd(nc, in_maps, core_ids=list(range(NCORES)))
    return _unshard([res.results[c]["out"] for c in range(NCORES)])
